# revision 11
# baseline (speedup 1.0000x reference)
"""Trainium2 Bass kernel for nn_DIDAModuleD4 (dynamic depthwise conv module).

Data-parallel over batch: 32 samples -> 8 cores x 4 samples.
Per core, samples are processed in 2 blocks of 2 samples; each block maps the
2x64=128 (sample, channel) pairs onto the 128 SBUF partitions.

Math (per sample, with host-side weight folding):
  f   = relu(conv_w @ x + conv_b)                       [64, 4096]
  g   = relu(mean_px(conv_w @ x + conv_b))              [64]
  k_t = a_t * g + b_t            (43 taps, a/b host-folded scalars)
  o_i = sum_t k_t * shift_t(f)   (depthwise; 5x5, 3x3 d2, 3x3 d4)
  out = sum_i (W_i diag(g-fold)) @ o_i + bias_out       [384, 4096]
        where W_i = fc_w[:, 128i:128(i+1)] @ fuse_w  (host-folded)

Engine split: 5x5 branch = 25 diagonal-matmul PSUM accumulations on TensorE;
3x3 branches split between DVE and GPSIMD scalar_tensor_tensor accumulators.
All matmul operands are float32r (full-rate PE, ~1e-4 rel err).
"""

import sys

if "/opt/trn_rl_repo" not in sys.path:
    sys.path.insert(0, "/opt/trn_rl_repo")

import os
import numpy as np
from contextlib import ExitStack

from concourse import bass, mybir, tile, bacc
from concourse.bass_utils import run_bass_kernel_spmd

DEBUG = bool(int(os.environ.get("BASSK_DEBUG", "0")))

F32 = mybir.dt.float32
F32R = mybir.dt.float32r
AF = mybir.ActivationFunctionType
ALU = mybir.AluOpType

N_CORES = 8
SAMPLES_PER_CORE = 4
CM = 64          # reduced channels / groups
CIN = 256
COUT = 384
H = W = 64
PIX = H * W      # 4096
PAD = 4
WP = W + 2 * PAD  # 72
SLAB = 1024      # pixels per processing slab (quarter of an image)
NSLAB = PIX // SLAB          # 4
CHUNK = 512                  # matmul N (one PSUM bank)
NCHUNK = PIX // CHUNK        # 8

# taps: (branch, dy, dx, dilation); ktile column order must match aT/bT
TAPS = (
    [(0, dy, dx, 1) for dy in range(-2, 3) for dx in range(-2, 3)]
    + [(1, dy, dx, 2) for dy in range(-1, 2) for dx in range(-1, 2)]
    + [(2, dy, dx, 4) for dy in range(-1, 2) for dx in range(-1, 2)]
)
NTAP = len(TAPS)  # 43

# engine assignment: branch0 (25 taps) -> TensorE; branches 1,2 split DVE/GPS
TENSOR_TAPS = [t for t in range(NTAP) if TAPS[t][0] == 0]
_B1 = [t for t in range(NTAP) if TAPS[t][0] == 1]
_B2 = [t for t in range(NTAP) if TAPS[t][0] == 2]
GPS_TAPS = _B2[:4]          # GPSIMD takes 4 of the dil-4 taps
DVE_TAPS = _B1 + _B2[4:]    # DVE takes the rest (14)

_PROGRAM_CACHE = {}


def _fpad_view(fp_t, r0, nrows, off_r, off_c, dtype=None):
    """View of padded-f tile [128, WP*WP] covering output rows [r0, r0+nrows)
    shifted by (off_r, off_c). Returns [128, nrows, 64] AP."""
    v = fp_t[:].rearrange("p (r c) -> p r c", c=WP)
    if dtype is not None:
        v = v.bitcast(dtype)
    return v[:, PAD + r0 + off_r : PAD + r0 + nrows + off_r,
             PAD + off_c : PAD + W + off_c]


def _build_program():
    nc = bacc.Bacc("TRN2", target_bir_lowering=False, debug=False,
                   num_devices=N_CORES)

    x4 = nc.dram_tensor("x4", [SAMPLES_PER_CORE, CIN, PIX], F32,
                        kind="ExternalInput").ap()
    wconv = nc.dram_tensor("wconv", [4, 128, 128], F32,
                           kind="ExternalInput").ap()
    wout = nc.dram_tensor("wout", [128, 3 * COUT], F32,
                          kind="ExternalInput").ap()
    aT_d = nc.dram_tensor("aT", [128, NTAP], F32, kind="ExternalInput").ap()
    bT_d = nc.dram_tensor("bT", [128, NTAP], F32, kind="ExternalInput").ap()
    ident_d = nc.dram_tensor("ident", [128, 128], F32,
                             kind="ExternalInput").ap()
    convb_d = nc.dram_tensor("convb", [128, 1], F32, kind="ExternalInput").ap()
    biasout_d = nc.dram_tensor("biasout", [128, 3], F32,
                               kind="ExternalInput").ap()
    y4 = nc.dram_tensor("y4", [SAMPLES_PER_CORE, COUT, PIX], F32,
                        kind="ExternalOutput").ap()
    if DEBUG:
        dbg_fpad = nc.dram_tensor("dbg_fpad", [128, WP * WP], F32,
                                  kind="ExternalOutput").ap()
        dbg_g = nc.dram_tensor("dbg_g", [128, 1 + NTAP], F32,
                               kind="ExternalOutput").ap()
        dbg_o = nc.dram_tensor("dbg_o", [4, 128, PIX], F32,
                               kind="ExternalOutput").ap()

    with tile.TileContext(nc) as tc:
        with ExitStack() as ctx:
            consts = ctx.enter_context(tc.tile_pool(name="consts", bufs=1))
            xpool = ctx.enter_context(tc.tile_pool(name="xp", bufs=2))
            fpool = ctx.enter_context(tc.tile_pool(name="fp", bufs=1))
            opool = ctx.enter_context(tc.tile_pool(name="op", bufs=2))
            outpool = ctx.enter_context(tc.tile_pool(name="outp", bufs=2))
            smalls = ctx.enter_context(tc.tile_pool(name="sm", bufs=2))
            diagp = ctx.enter_context(tc.tile_pool(name="dg", bufs=2))
            ps_conv = ctx.enter_context(
                tc.tile_pool(name="psc", bufs=2, space="PSUM"))
            ps_o1 = ctx.enter_context(
                tc.tile_pool(name="ps1", bufs=2, space="PSUM"))
            ps_out0 = ctx.enter_context(
                tc.tile_pool(name="pso0", bufs=2, space="PSUM"))
            ps_out1 = ctx.enter_context(
                tc.tile_pool(name="pso1", bufs=2, space="PSUM"))

            # ---- constants (loaded once) ----
            wconv_t = consts.tile([128, 4 * 128], F32R, tag="wconv")
            for g_ in range(4):
                nc.sync.dma_start(wconv_t[:, g_ * 128:(g_ + 1) * 128],
                                  wconv[g_].bitcast(F32R))
            wout_t = consts.tile([128, 3 * COUT], F32R, tag="wout")
            nc.sync.dma_start(wout_t[:], wout[:].bitcast(F32R))
            aT = consts.tile([128, NTAP], F32, tag="aT")
            nc.sync.dma_start(aT[:], aT_d[:])
            bT = consts.tile([128, NTAP], F32, tag="bT")
            nc.sync.dma_start(bT[:], bT_d[:])
            ident = consts.tile([128, 128], F32, tag="ident")
            nc.sync.dma_start(ident[:], ident_d[:])
            convb = consts.tile([128, 1], F32, tag="convb")
            nc.sync.dma_start(convb[:], convb_d[:])
            biasout = consts.tile([128, 3], F32, tag="biasout")
            nc.sync.dma_start(biasout[:], biasout_d[:])

            # persistent padded-f tiles (one per block parity); borders are
            # zeroed once and never rewritten (interior writes only).
            # memset cannot emit float32r, so zero an f32 scratch and copy
            # with dtype conversion (the copy is the f32r-rounded producer).
            zeros = consts.tile([128, PAD * WP], F32, tag="zeros")
            nc.gpsimd.memset(zeros[:], 0.0)
            fpads = []
            for par in range(2):
                fp_t = fpool.tile([128, WP * WP], F32R, tag=f"fpad{par}")
                v = fp_t[:].rearrange("p (r c) -> p r c", c=WP)
                nc.vector.tensor_copy(fp_t[:, 0:PAD * WP], zeros[:])
                nc.vector.tensor_copy(fp_t[:, (PAD + H) * WP:WP * WP],
                                      zeros[:])
                zv = zeros[:].rearrange("p (r c) -> p r c", c=PAD)
                nc.vector.tensor_copy(v[:, PAD:PAD + H, 0:PAD],
                                      zv[:, 0:H, :])
                nc.vector.tensor_copy(v[:, PAD:PAD + H, PAD + W:WP],
                                      zv[:, 0:H, :])
                fpads.append(fp_t)

            for blk in range(SAMPLES_PER_CORE // 2):
                n0, n1 = 2 * blk, 2 * blk + 1
                fp_t = fpads[blk % 2]

                # ---- conv 1x1 + g accumulation ----
                gsums = smalls.tile([128, NCHUNK], F32, tag="gsums")
                for q in range(NSLAB):
                    # x slab tiles: 4 K-groups, each [128, SLAB] f32r
                    xts = []
                    for g_ in range(4):
                        xt = xpool.tile([128, SLAB], F32R, tag=f"x{g_}")
                        for s, n in enumerate((n0, n1)):
                            nc.sync.dma_start(
                                xt[64 * s:64 * s + 64, :],
                                x4[n, 64 * g_:64 * g_ + 64,
                                   q * SLAB:(q + 1) * SLAB].bitcast(F32R))
                        xts.append(xt)
                    for c in range(SLAB // CHUNK):
                        j = q * (SLAB // CHUNK) + c  # global chunk index
                        psf = ps_conv.tile([128, CHUNK], F32, tag="convps")
                        for g_ in range(4):
                            nc.tensor.matmul(
                                psf[:],
                                wconv_t[:, g_ * 128:(g_ + 1) * 128],
                                xts[g_][:, c * CHUNK:(c + 1) * CHUNK],
                                start=(g_ == 0), stop=(g_ == 3))
                        # evacuate with bias; accum_out gives sum for g
                        dst = _fpad_view(fp_t, 8 * j, 8, 0, 0)
                        nc.scalar.activation(dst, psf[:], AF.Identity,
                                             bias=convb[:, 0:1],
                                             accum_out=gsums[:, j:j + 1])

                # in-place relu over the interior
                intr = _fpad_view(fp_t, 0, H, 0, 0)
                nc.vector.tensor_scalar_max(intr, intr.bitcast(F32), 0.0)

                # ---- g, ktile, scaled output weights, diag tiles ----
                gpre = smalls.tile([128, 1], F32, tag="gpre")
                nc.vector.tensor_reduce(gpre[:], gsums[:], op=ALU.add,
                                        axis=mybir.AxisListType.X)
                gt = smalls.tile([128, 1], F32, tag="g")
                nc.scalar.activation(gt[:], gpre[:], AF.Relu,
                                     scale=1.0 / PIX)
                ktile = smalls.tile([128, NTAP], F32, tag="ktile")
                nc.vector.scalar_tensor_tensor(ktile[:], aT[:], gt[:, 0:1],
                                               bT[:], op0=ALU.mult,
                                               op1=ALU.add)

                diags = {}
                for t in TENSOR_TAPS:
                    dg = diagp.tile([128, 128], F32R, tag=f"diag{t}")
                    nc.gpsimd.tensor_scalar_mul(dg[:], ident[:],
                                                ktile[:, t:t + 1])
                    diags[t] = dg

                # ---- taps + output matmul, per slab ----
                for q in range(NSLAB):
                    r0 = q * (SLAB // W)      # first output row of slab
                    nr = SLAB // W            # rows per slab (16)

                    # branch 0 on TensorE: diag-matmul accumulation per chunk
                    o1_t = opool.tile([128, SLAB], F32R, tag="o1")
                    for c in range(SLAB // CHUNK):
                        pso = ps_o1.tile([128, CHUNK], F32, tag="o1ps")
                        for i, t in enumerate(TENSOR_TAPS):
                            _, dy, dx, dil = TAPS[t]
                            rhs = _fpad_view(fp_t, r0 + c * (CHUNK // W),
                                             CHUNK // W, dy * dil, dx * dil)
                            nc.tensor.matmul(pso[:], diags[t][:], rhs,
                                             start=(i == 0),
                                             stop=(i == len(TENSOR_TAPS) - 1))
                        nc.scalar.activation(
                            o1_t[:, c * CHUNK:(c + 1) * CHUNK], pso[:],
                            AF.Copy)

                    # branches 1,2 on DVE / GPSIMD
                    acc_tiles = {}  # branch -> (tile, weight_branch)
                    o2_t = opool.tile([128, SLAB], F32R, tag="o2")
                    o3_t = opool.tile([128, SLAB], F32R, tag="o3")
                    dve_by_branch = {1: [], 2: []}
                    for t in DVE_TAPS:
                        dve_by_branch[TAPS[t][0]].append(t)
                    for br, ot in ((1, o2_t), (2, o3_t)):
                        for i, t in enumerate(dve_by_branch[br]):
                            _, dy, dx, dil = TAPS[t]
                            src = _fpad_view(fp_t, r0, nr, dy * dil, dx * dil,
                                             dtype=F32)
                            ov = ot[:].rearrange("p (r c) -> p r c", c=W)
                            if i == 0:
                                nc.vector.tensor_scalar_mul(
                                    ov, src, ktile[:, t:t + 1])
                            else:
                                nc.vector.scalar_tensor_tensor(
                                    ov, src, ktile[:, t:t + 1],
                                    ov.bitcast(F32), op0=ALU.mult,
                                    op1=ALU.add)
                    og_t = None
                    if GPS_TAPS:
                        # GPSIMD has no scalar_tensor_tensor: use
                        # tensor_scalar into tmp + tensor_tensor accumulate.
                        og_t = opool.tile([128, SLAB], F32R, tag="og")
                        ov = og_t[:].rearrange("p (r c) -> p r c", c=W)
                        for i, t in enumerate(GPS_TAPS):
                            _, dy, dx, dil = TAPS[t]
                            src = _fpad_view(fp_t, r0, nr, dy * dil, dx * dil,
                                             dtype=F32)
                            if i == 0:
                                nc.gpsimd.tensor_scalar_mul(
                                    ov, src, ktile[:, t:t + 1])
                            else:
                                tmp = opool.tile([128, SLAB], F32,
                                                 tag="ogtmp")
                                tv = tmp[:].rearrange("p (r c) -> p r c", c=W)
                                nc.gpsimd.tensor_scalar_mul(
                                    tv, src, ktile[:, t:t + 1])
                                nc.gpsimd.tensor_tensor(
                                    out=ov, in0=ov.bitcast(F32),
                                    in1=tv, op=ALU.add)

                    if DEBUG and blk == 0:
                        sl = (q * SLAB, (q + 1) * SLAB)
                        nc.sync.dma_start(dbg_o[0, :, sl[0]:sl[1]],
                                          o1_t[:].bitcast(F32))
                        nc.sync.dma_start(dbg_o[1, :, sl[0]:sl[1]],
                                          o2_t[:].bitcast(F32))
                        nc.sync.dma_start(dbg_o[2, :, sl[0]:sl[1]],
                                          o3_t[:].bitcast(F32))
                        if og_t is not None:
                            nc.sync.dma_start(dbg_o[3, :, sl[0]:sl[1]],
                                              og_t[:].bitcast(F32))
                        if q == NSLAB - 1:
                            nc.sync.dma_start(dbg_fpad[:],
                                              fp_t[:].bitcast(F32))
                            nc.sync.dma_start(dbg_g[:, 0:1], gt[:])
                            nc.sync.dma_start(dbg_g[:, 1:1 + NTAP], ktile[:])

                    # output matmul: pieces (acc tile, branch weight)
                    pieces = [(o1_t, 0), (o2_t, 1), (o3_t, 2)]
                    if og_t is not None:
                        pieces.append((og_t, 2))
                    osbs = {}
                    for mt in range(3):
                        for s in range(2):
                            osb_tile = outpool.tile([128, SLAB], F32,
                                                    tag=f"osb{mt}_{s}")
                            osbs[(mt, s)] = osb_tile
                    for c in range(SLAB // CHUNK):
                        for mt in range(3):
                            pss = []
                            for s, psp in ((0, ps_out0), (1, ps_out1)):
                                ps = psp.tile([128, CHUNK], F32,
                                              tag=f"outps{s}")
                                pss.append(ps)
                                for ip, (ot, br) in enumerate(pieces):
                                    lhsT = wout_t[64 * s:64 * s + 64,
                                                  br * COUT + mt * 128:
                                                  br * COUT + (mt + 1) * 128]
                                    rhs = ot[64 * s:64 * s + 64,
                                             c * CHUNK:(c + 1) * CHUNK]
                                    nc.tensor.matmul(
                                        ps[:], lhsT, rhs,
                                        start=(ip == 0),
                                        stop=(ip == len(pieces) - 1))
                            for s in range(2):
                                osb = osbs[(mt, s)]
                                nc.scalar.activation(
                                    osb[:, c * CHUNK:(c + 1) * CHUNK],
                                    pss[s][:], AF.Identity,
                                    bias=biasout[:, mt:mt + 1])
                                if c == SLAB // CHUNK - 1:
                                    n = (n0, n1)[s]
                                    nc.sync.dma_start(
                                        y4[n, mt * 128:(mt + 1) * 128,
                                           q * SLAB:(q + 1) * SLAB],
                                        osb[:])
    nc.compile()
    return nc


def _get_program():
    if "nc" not in _PROGRAM_CACHE:
        _PROGRAM_CACHE["nc"] = _build_program()
    return _PROGRAM_CACHE["nc"]


def kernel(x, conv_w, conv_b, ck_w, ck_b, ck2_w, ck2_b, ckd4_w, ckd4_b,
           kern_w, kern_b, kern2_w, kern2_b, kernd4_w, kernd4_b,
           fuse_w, fuse_b, fc_w, fc_b):
    x = np.asarray(x, dtype=np.float32)
    conv_w = np.asarray(conv_w, dtype=np.float32)
    conv_b = np.asarray(conv_b, dtype=np.float32)
    fuse_w = np.asarray(fuse_w, dtype=np.float32)
    fuse_b = np.asarray(fuse_b, dtype=np.float32)
    fc_w = np.asarray(fc_w, dtype=np.float32)
    fc_b = np.asarray(fc_b, dtype=np.float32)

    NB = x.shape[0]
    assert NB == N_CORES * SAMPLES_PER_CORE

    # ---- host-side weight folding ----
    # tap affine coefficients: k_t = a_t * g + b_t
    a1 = (float(ck_w) * np.asarray(kern_w)).astype(np.float32)        # [25]
    b1 = (float(ck_w) * np.asarray(kern_b) + float(ck_b)).astype(np.float32)
    a2 = (float(ck2_w) * np.asarray(kern2_w)).astype(np.float32)      # [9]
    b2 = (float(ck2_w) * np.asarray(kern2_b) + float(ck2_b)).astype(np.float32)
    a3 = (float(ckd4_w) * np.asarray(kernd4_w)).astype(np.float32)    # [9]
    b3 = (float(ckd4_w) * np.asarray(kernd4_b) + float(ckd4_b)).astype(np.float32)
    a_all = np.concatenate([a1, a2, a3]).astype(np.float32)           # [43]
    b_all = np.concatenate([b1, b2, b3]).astype(np.float32)
    aT = np.broadcast_to(a_all, (128, NTAP)).copy()
    bT = np.broadcast_to(b_all, (128, NTAP)).copy()

    # folded output weights W_i = fc_w[:, 128i:128(i+1)] @ fuse_w  [384, 64]
    Wi = [fc_w[:, 128 * i:128 * (i + 1)] @ fuse_w for i in range(3)]
    wout = np.zeros((128, 3 * COUT), dtype=np.float32)
    for i in range(3):
        wt = Wi[i].T.astype(np.float32)           # [64, 384]
        wout[0:64, i * COUT:(i + 1) * COUT] = wt
        wout[64:128, i * COUT:(i + 1) * COUT] = wt
    bias_out = (fc_w @ np.tile(fuse_b, 3) + fc_b).astype(np.float32)  # [384]
    biasout = bias_out.reshape(3, 128).T.copy()   # [128, 3], col mt

    # block-diagonal conv lhsT per K-group
    wconv = np.zeros((4, 128, 128), dtype=np.float32)
    for g_ in range(4):
        blkw = conv_w[:, 64 * g_:64 * g_ + 64].T  # [64, 64] = lhsT block
        wconv[g_, 0:64, 0:64] = blkw
        wconv[g_, 64:128, 64:128] = blkw

    convb = np.concatenate([conv_b, conv_b]).reshape(128, 1).astype(np.float32)
    ident = np.eye(128, dtype=np.float32)

    nc = _get_program()
    in_maps = []
    for core in range(N_CORES):
        xs = x[core * SAMPLES_PER_CORE:(core + 1) * SAMPLES_PER_CORE]
        in_maps.append({
            "x4": np.ascontiguousarray(xs.reshape(SAMPLES_PER_CORE, CIN, PIX)),
            "wconv": wconv, "wout": wout, "aT": aT, "bT": bT,
            "ident": ident, "convb": convb, "biasout": biasout,
        })
    res = run_bass_kernel_spmd(nc, in_maps, list(range(N_CORES)))
    out = np.empty((NB, COUT, H, W), dtype=np.float32)
    for core in range(N_CORES):
        out[core * SAMPLES_PER_CORE:(core + 1) * SAMPLES_PER_CORE] = (
            res.results[core]["y4"].reshape(SAMPLES_PER_CORE, COUT, H, W))
    return out
